# revision 10
# baseline (speedup 1.0000x reference)
"""AIS estimator (nn_AIS_Estimator) on 8 Trainium2 NeuronCores.

Math: the reference's leapfrog gradient is affine, g(q) = C_k - q @ (b_k W W^T + I),
so each annealing step collapses (on host, f64) to an affine map
    q <- q @ A_k + n_k @ B_k + D_k        (A_k, B_k 64x64; D_k folded into the
                                           noise: n'_k = n_k + D_k @ inv(B_k))
The incremental weight is
    w(q) = -0.5*DX*LOG2PI - 0.5*||q W - x||^2 - q.mu + 0.5*||mu||^2
(prior - importance collapses to the bilinear -q.mu + const). The k=0 term
uses the raw inputs and is computed on host; steps 1..K run on device:
    psum_q = q @ A2_k + n' @ B2_k                      (PE)
    q      = copy(psum_q)                              (DVE)
    psum_r = q @ W2 + (-x) via identity-tile matmul    (PE)
    rr     = Square(sqrt(db_k) * psum_r)               (ACT)
    qm     = q * muT                                   (GPSIMD)
    slw   += reduce_partitions(rr, -0.5) + reduce(qm, -db_k)   (PE, rr/qm as
             stationary operand, [128,2] out per 128-col sub-tile into a
             single [128,128] PSUM accumulator)
Momentum refresh noise is input-independent (fold_in(key42, k)) and
precomputed on host CPU via JAX threefry.

Device layout per core (128 of 1024 samples, pure data parallel):
  partitions = (half, d) [2*64], free = (s', b) [64*128 = 8192]; 64-dim
  matmuls use all 128 partitions via block-diagonal weights. Host gathers
  per-core [128,128] slw partials, adds constants, does logsumexp over n.
"""

import math
import os

import numpy as np

N, B, D, DX = 1024, 128, 64, 64
K, NL, H = 16, 3, 0.05
NCORES = 8
NLOC = N // NCORES          # samples per core
SP = NLOC // 2              # s' per half
FREE = SP * B               # 8192
CHUNK = 512
NCH = FREE // CHUNK         # 16
LOG2PI = math.log(2.0 * math.pi)

_cache = {}


def _betas():
    z = 4.0 * (2.0 * np.linspace(0.0, 1.0, K + 2) - 1.0)
    b = 1.0 / (1.0 + np.exp(-z))
    return (b - b[0]) / (b[-1] - b[0])  # f64, [K+2]


def _layout(arr):
    """(N,B,D) f32 -> [NCORES, 128, FREE] device layout [p=(h,d), f=(s',b)]."""
    a = arr.reshape(NCORES, 2, SP, B, D).transpose(0, 1, 4, 2, 3)
    return np.ascontiguousarray(a.reshape(NCORES, 2 * D, FREE))


def _off_tile(off):
    """(B,D) f64 offset -> [128, FREE] f32 tile broadcast over s'."""
    t = np.empty((2 * D, B), np.float32)
    t[:D] = off.T.astype(np.float32)
    t[D:] = off.T.astype(np.float32)
    return np.tile(t, (1, SP))


def _bd(m):
    """(D,D) -> [128,128] f32 block-diag (two sample-halves)."""
    r, c = m.shape
    out = np.zeros((2 * r, 2 * c), np.float32)
    out[:r, :c] = m
    out[r:, c:] = m
    return out


def _dup_cols(m):
    """(B, D) -> [128, 128] f32: columns duplicated for both halves."""
    out = np.empty((B, 2 * m.shape[1]), np.float32)
    out[:, : m.shape[1]] = m
    out[:, m.shape[1] :] = m
    return out


def _refresh_noise():
    # Momentum refresh noise: jax.random.normal(fold_in(key(42), k), ...).
    # Generated on the DEFAULT jax backend: with jax_default_prng_impl=rbg the
    # bits are backend-dependent, and the graded reference runs on the default
    # backend, so we must sample from the same generator.
    if "nz" not in _cache:
        import jax
        import jax.numpy as jnp

        gen = jax.jit(
            lambda k: jax.random.normal(
                jax.random.fold_in(jax.random.key(42), k), (N, B, D), jnp.float32
            )
        )
        out = [_layout(np.asarray(gen(jnp.int32(k)))) for k in range(1, K + 1)]
        _cache["nz"] = out
    return _cache["nz"]


def host_maps(x, W_enc, W_dec):
    """Input-dependent f64 precomputation of the per-step affine maps."""
    xd = np.asarray(x, np.float64)
    Wed = np.asarray(W_enc, np.float64)
    Wdd = np.asarray(W_dec, np.float64)
    beta = _betas()
    mu = xd @ Wed                      # (B,D)
    A = xd @ Wdd.T                     # (B,D)
    M = Wdd @ Wdd.T
    I = np.eye(D)

    was, wbs, offs = [], [], []
    for k in range(1, K + 1):
        bk = beta[k]
        Mk = bk * M + I
        Ck = bk * A + (1.0 - bk) * mu
        al, be = I.copy(), np.zeros((D, D))
        ga, ep = np.zeros((D, D)), H * I
        de, ze = np.zeros((B, D)), np.zeros((B, D))
        for j in range(NL):
            s = H * (H / 2 if j == 0 else H)
            ga = ga - s * (al @ Mk)
            ep = ep - s * (be @ Mk)
            ze = ze + s * Ck - s * (de @ Mk)
            al = al + ga
            be = be + ep
            de = de + ze
        was.append(al)
        wbs.append(be)
        offs.append(de @ np.linalg.inv(be))

    return dict(was=was, wbs=wbs, offs=offs, mu=mu, W=Wdd, beta=beta)


def _program():
    if "nc" in _cache:
        return _cache["nc"]
    import concourse.bacc as bacc
    import concourse.mybir as mybir
    from concourse import tile

    f32 = mybir.dt.float32
    SQ = mybir.ActivationFunctionType.Square
    beta = _betas()
    db = beta[1:] - beta[:-1]

    nc = bacc.Bacc()
    qn = nc.declare_dram_parameter("qn", [128, FREE], f32, isOutput=False)
    nz = [
        nc.declare_dram_parameter(f"nz{k}", [128, FREE], f32, isOutput=False)
        for k in range(1, K + 1)
    ]
    wa = [
        nc.declare_dram_parameter(f"wa{k}", [128, 128], f32, isOutput=False)
        for k in range(1, K + 1)
    ]
    wb = [
        nc.declare_dram_parameter(f"wb{k}", [128, 128], f32, isOutput=False)
        for k in range(1, K + 1)
    ]
    ww = nc.declare_dram_parameter("ww", [128, 128], f32, isOutput=False)
    wx = nc.declare_dram_parameter("wx", [128, 128], f32, isOutput=False)
    wi = nc.declare_dram_parameter("wi", [128, CHUNK], f32, isOutput=False)
    mut = nc.declare_dram_parameter("mut", [128, CHUNK], f32, isOutput=False)
    redm = nc.declare_dram_parameter("redm", [128, 2], f32, isOutput=False)
    redq = nc.declare_dram_parameter("redq", [128, 2 * K], f32, isOutput=False)
    out = nc.declare_dram_parameter("out", [128, 2 * SP], f32, isOutput=True)

    with tile.TileContext(nc) as tc:
        with (
            tc.tile_pool(name="wts", bufs=1) as wp,
            tc.tile_pool(name="state", bufs=1) as sp,
            tc.tile_pool(name="noise", bufs=2) as npl,
            tc.tile_pool(name="rr", bufs=4) as rp,
            tc.tile_pool(name="ps", bufs=3, space="PSUM") as pp,
            tc.tile_pool(name="slw", bufs=1, space="PSUM") as lp,
        ):
            def wtile(src, shape, tag):
                t = wp.tile(shape, f32, tag=tag)
                nc.sync.dma_start(t[:], src[:])
                return t

            ww_t = wtile(ww, [128, 128], "ww")
            wx_t = wtile(wx, [128, 128], "wx")
            wi_t = wtile(wi, [128, CHUNK], "wi")
            mut_t = wtile(mut, [128, CHUNK], "mut")
            redm_t = wtile(redm, [128, 2], "redm")
            redq_t = wtile(redq, [128, 2 * K], "redq")
            wa_t = [wtile(wa[k], [128, 128], f"wa{k}") for k in range(K)]
            wb_t = [wtile(wb[k], [128, 128], f"wb{k}") for k in range(K)]

            q = sp.tile([128, FREE], f32, tag="q")
            slw = lp.tile([128, 2 * SP], f32, tag="slw")
            # Reduce matmuls accumulate with start=False throughout: a
            # start=True would mark the whole 2KB bank pending-zero and wipe
            # sibling column-pairs. Zero the accumulator explicitly instead.
            nc.vector.memset(slw[:], 0.0)

            def dma_in(dst, src):
                step = FREE // 4
                for j in range(4):
                    sl = slice(j * step, (j + 1) * step)
                    nc.sync.dma_start(dst[:, sl], src[:, sl])

            # q0 arrives via DRAM; its w-term is handled on host.
            dma_in(q, qn)

            for k in range(K):
                nz_t = npl.tile([128, FREE], f32, tag="nz")
                dma_in(nz_t, nz[k])
                sk = float(np.sqrt(db[k + 1]))
                for ci in range(NCH):
                    sl = slice(ci * CHUNK, (ci + 1) * CHUNK)
                    psq = pp.tile([128, CHUNK], f32, tag="psq")
                    nc.tensor.matmul(psq[:], wa_t[k][:], q[:, sl], start=True, stop=False)
                    nc.tensor.matmul(psq[:], wb_t[k][:], nz_t[:, sl], start=False, stop=True)
                    nc.vector.tensor_copy(q[:, sl], psq[:])

                    psr = pp.tile([128, CHUNK], f32, tag="psr")
                    nc.tensor.matmul(psr[:], ww_t[:], q[:, sl], start=True, stop=False)
                    nc.tensor.matmul(psr[:], wx_t[:], wi_t[:], start=False, stop=True)
                    rr = rp.tile([128, CHUNK], f32, tag="rr")
                    nc.scalar.activation(rr[:], psr[:], SQ, scale=sk)

                    qm = rp.tile([128, CHUNK], f32, tag="qm")
                    nc.gpsimd.tensor_mul(qm[:], q[:, sl], mut_t[:])

                    for sub in range(4):
                        col = (ci * 4 + sub) * 2
                        ssl = slice(sub * 128, (sub + 1) * 128)
                        nc.tensor.matmul(
                            slw[:, col : col + 2], rr[:, ssl], redm_t[:],
                            start=False, stop=False, skip_group_check=True,
                        )
                        nc.tensor.matmul(
                            slw[:, col : col + 2], qm[:, ssl],
                            redq_t[:, 2 * k : 2 * k + 2],
                            start=False, stop=(k == K - 1), skip_group_check=True,
                        )

            out_t = wp.tile([128, 2 * SP], f32, tag="out")
            nc.vector.tensor_copy(out_t[:], slw[:])
            nc.sync.dma_start(out[:], out_t[:])
    nc.compile()
    _cache["nc"] = nc
    return nc


def build_in_maps(x, W_enc, W_dec, q_noise):
    x = np.asarray(x, np.float32)
    hm = host_maps(x, W_enc, W_dec)
    beta, mu, W = hm["beta"], hm["mu"], hm["W"]
    db = beta[1:] - beta[:-1]
    nzl = _refresh_noise()
    q_noise = np.asarray(q_noise, np.float32)

    # q0 = mu + q_noise, in device layout
    q0l = _layout(q_noise) + _off_tile(mu)[None]
    offts = [_off_tile(o) for o in hm["offs"]]

    muT = np.empty((2 * D, B), np.float32)
    muT[:D] = mu.T.astype(np.float32)
    muT[D:] = mu.T.astype(np.float32)
    mut_np = np.tile(muT, (1, CHUNK // B))           # [128, 512]

    eye = np.eye(B, dtype=np.float32)
    wi_np = np.tile(eye, (1, CHUNK // B))            # [128, 512]
    wx_np = _dup_cols(-x)                            # [128, 128]
    ww_np = _bd(W.astype(np.float32))

    half = np.zeros((128, 2), np.float32)
    half[:D, 0] = 1.0
    half[D:, 1] = 1.0
    redm_np = -0.5 * half
    redq_np = np.concatenate(
        [(-db[k + 1]) * half for k in range(K)], axis=1
    ).astype(np.float32)                             # [128, 2K]

    wa_np = [_bd(a.astype(np.float32)) for a in hm["was"]]
    wb_np = [_bd(b.astype(np.float32)) for b in hm["wbs"]]

    in_maps = []
    for c in range(NCORES):
        m = {
            "qn": q0l[c],
            "ww": ww_np, "wx": wx_np, "wi": wi_np, "mut": mut_np,
            "redm": redm_np, "redq": redq_np,
        }
        for k in range(K):
            m[f"nz{k + 1}"] = nzl[k][c] + offts[k]
            m[f"wa{k + 1}"] = wa_np[k]
            m[f"wb{k + 1}"] = wb_np[k]
        in_maps.append(m)

    # host-side constants: k=0 incremental weight + per-step const terms
    q0 = mu[None].astype(np.float32) + q_noise       # (N,B,D)
    r0 = q0.reshape(-1, D).astype(np.float32) @ W.astype(np.float32)
    r0 = r0.reshape(N, B, DX) - x[None]
    w0 = (
        -0.5 * DX * LOG2PI
        - 0.5 * np.einsum("nbe,nbe->nb", r0, r0, dtype=np.float64)
        - np.einsum("nbd,bd->nb", q0, mu.astype(np.float32), dtype=np.float64)
        + 0.5 * (mu * mu).sum(-1)[None]
    )
    const_nb = db[0] * w0                            # (N,B) f64
    rest = beta[K + 1] - beta[1]                     # sum of db[1:]
    const_b = rest * (-0.5 * DX * LOG2PI + 0.5 * (mu * mu).sum(-1))  # (B,)
    const = (const_nb + const_b[None]).astype(np.float32)
    return in_maps, const


def decode_out(dev, c, slw_all):
    """dev [128=b, 128=(s',h)] -> slw_all[c*NLOC:(c+1)*NLOC, :] (NLOC,B)."""
    a = dev.reshape(B, SP, 2).transpose(2, 1, 0)  # [h, s', b]
    slw_all[c * NLOC : (c + 1) * NLOC] = a.reshape(NLOC, B)


def kernel(x, W_enc, W_dec, q_noise, p_noise):
    from concourse.bass_utils import run_bass_kernel_spmd

    nc = _program()
    in_maps, const = build_in_maps(x, W_enc, W_dec, q_noise)
    trace = bool(int(os.environ.get("KERNEL_TRACE", "0")))
    res = run_bass_kernel_spmd(nc, in_maps, list(range(NCORES)), trace=trace)
    _cache["last_result"] = res

    slw = np.empty((N, B), np.float32)
    for c in range(NCORES):
        decode_out(np.asarray(res.results[c]["out"]), c, slw)
    slw += const
    m = slw.max(0)
    out = m + np.log(np.exp(slw - m).sum(0, dtype=np.float32)) - math.log(float(N))
    return out.astype(np.float32)


# revision 11
# speedup vs baseline: 1.1736x; 1.1736x over previous
"""AIS estimator (nn_AIS_Estimator) on 8 Trainium2 NeuronCores.

Math: the reference's leapfrog gradient is affine, g(q) = C_k - q @ (b_k W W^T + I),
so each annealing step collapses (on host, f64) to an affine map
    q <- q @ A_k + n_k @ B_k + D_k        (A_k, B_k 64x64; D_k folded into the
                                           noise: n'_k = n_k + D_k @ inv(B_k))
The incremental weight is
    w(q) = -0.5*DX*LOG2PI - 0.5*||q W - x||^2 - q.mu + 0.5*||mu||^2
(prior - importance collapses to the bilinear -q.mu + const). The k=0 term
uses the raw inputs and is computed on host; steps 1..K run on device:
    psum_q = q @ A2_k + n' @ B2_k                      (PE)
    q      = copy(psum_q)                              (DVE)
    psum_r = q @ W2 + (-x) via identity-tile matmul    (PE)
    rr     = Square(sqrt(db_k) * psum_r)               (ACT)
    qm     = q * muT                                   (GPSIMD)
    slw   += reduce_partitions(rr, -0.5) + reduce(qm, -db_k)   (PE, rr/qm as
             stationary operand, [128,2] out per 128-col sub-tile into a
             single [128,128] PSUM accumulator)
Momentum refresh noise is input-independent (fold_in(key42, k)) and
precomputed on host CPU via JAX threefry.

Device layout per core (128 of 1024 samples, pure data parallel):
  partitions = (half, d) [2*64], free = (s', b) [64*128 = 8192]; 64-dim
  matmuls use all 128 partitions via block-diagonal weights. Host gathers
  per-core [128,128] slw partials, adds constants, does logsumexp over n.
"""

import math
import os

import numpy as np

N, B, D, DX = 1024, 128, 64, 64
K, NL, H = 16, 3, 0.05
NCORES = 8
NLOC = N // NCORES          # samples per core
SP = NLOC // 2              # s' per half
FREE = SP * B               # 8192
CHUNK = 512
NCH = FREE // CHUNK         # 16
LOG2PI = math.log(2.0 * math.pi)

_cache = {}


def _betas():
    z = 4.0 * (2.0 * np.linspace(0.0, 1.0, K + 2) - 1.0)
    b = 1.0 / (1.0 + np.exp(-z))
    return (b - b[0]) / (b[-1] - b[0])  # f64, [K+2]


def _layout(arr):
    """(N,B,D) f32 -> [NCORES, 128, FREE] device layout [p=(h,d), f=(s',b)]."""
    a = arr.reshape(NCORES, 2, SP, B, D).transpose(0, 1, 4, 2, 3)
    return np.ascontiguousarray(a.reshape(NCORES, 2 * D, FREE))


def _off_tile(off):
    """(B,D) f64 offset -> [128, FREE] f32 tile broadcast over s'."""
    t = np.empty((2 * D, B), np.float32)
    t[:D] = off.T.astype(np.float32)
    t[D:] = off.T.astype(np.float32)
    return np.tile(t, (1, SP))


def _bd(m):
    """(D,D) -> [128,128] f32 block-diag (two sample-halves)."""
    r, c = m.shape
    out = np.zeros((2 * r, 2 * c), np.float32)
    out[:r, :c] = m
    out[r:, c:] = m
    return out


def _dup_cols(m):
    """(B, D) -> [128, 128] f32: columns duplicated for both halves."""
    out = np.empty((B, 2 * m.shape[1]), np.float32)
    out[:, : m.shape[1]] = m
    out[:, m.shape[1] :] = m
    return out


def _refresh_noise():
    # Momentum refresh noise: jax.random.normal(fold_in(key(42), k), ...).
    # Generated on the DEFAULT jax backend: with jax_default_prng_impl=rbg the
    # bits are backend-dependent, and the graded reference runs on the default
    # backend, so we must sample from the same generator.
    if "nz" not in _cache:
        import jax
        import jax.numpy as jnp

        gen = jax.jit(
            lambda k: jax.random.normal(
                jax.random.fold_in(jax.random.key(42), k), (N, B, D), jnp.float32
            )
        )
        out = [_layout(np.asarray(gen(jnp.int32(k)))) for k in range(1, K + 1)]
        _cache["nz"] = out
    return _cache["nz"]


def host_maps(x, W_enc, W_dec):
    """Input-dependent f64 precomputation of the per-step affine maps."""
    xd = np.asarray(x, np.float64)
    Wed = np.asarray(W_enc, np.float64)
    Wdd = np.asarray(W_dec, np.float64)
    beta = _betas()
    mu = xd @ Wed                      # (B,D)
    A = xd @ Wdd.T                     # (B,D)
    M = Wdd @ Wdd.T                    # A is reused for the q.E bilinear
    I = np.eye(D)

    was, wbs, offs = [], [], []
    for k in range(1, K + 1):
        bk = beta[k]
        Mk = bk * M + I
        Ck = bk * A + (1.0 - bk) * mu
        al, be = I.copy(), np.zeros((D, D))
        ga, ep = np.zeros((D, D)), H * I
        de, ze = np.zeros((B, D)), np.zeros((B, D))
        for j in range(NL):
            s = H * (H / 2 if j == 0 else H)
            ga = ga - s * (al @ Mk)
            ep = ep - s * (be @ Mk)
            ze = ze + s * Ck - s * (de @ Mk)
            al = al + ga
            be = be + ep
            de = de + ze
        was.append(al)
        wbs.append(be)
        offs.append(de @ np.linalg.inv(be))

    return dict(was=was, wbs=wbs, offs=offs, mu=mu, W=Wdd, beta=beta, A=A)


def _program():
    if "nc" in _cache:
        return _cache["nc"]
    import concourse.bacc as bacc
    import concourse.mybir as mybir
    from concourse import tile

    f32 = mybir.dt.float32
    SQ = mybir.ActivationFunctionType.Square
    beta = _betas()
    db = beta[1:] - beta[:-1]

    nc = bacc.Bacc()
    qn = nc.declare_dram_parameter("qn", [128, FREE], f32, isOutput=False)
    nz = [
        nc.declare_dram_parameter(f"nz{k}", [128, FREE], f32, isOutput=False)
        for k in range(1, K + 1)
    ]
    wa = [
        nc.declare_dram_parameter(f"wa{k}", [128, 128], f32, isOutput=False)
        for k in range(1, K + 1)
    ]
    wb = [
        nc.declare_dram_parameter(f"wb{k}", [128, 128], f32, isOutput=False)
        for k in range(1, K + 1)
    ]
    ww = nc.declare_dram_parameter("ww", [128, 128], f32, isOutput=False)
    et = nc.declare_dram_parameter("et", [128, CHUNK], f32, isOutput=False)
    redm = nc.declare_dram_parameter("redm", [128, 2], f32, isOutput=False)
    redq = nc.declare_dram_parameter("redq", [128, 2 * K], f32, isOutput=False)
    out = nc.declare_dram_parameter("out", [2, FREE], f32, isOutput=True)

    with tile.TileContext(nc) as tc:
        with (
            tc.tile_pool(name="wts", bufs=1) as wp,
            tc.tile_pool(name="state", bufs=1) as sp,
            tc.tile_pool(name="noise", bufs=2) as npl,
            tc.tile_pool(name="rr", bufs=4) as rp,
            tc.tile_pool(name="ps", bufs=3, space="PSUM") as pp,
            tc.tile_pool(name="psw", bufs=2, space="PSUM") as pw,
        ):
            def wtile(src, shape, tag):
                t = wp.tile(shape, f32, tag=tag)
                nc.sync.dma_start(t[:], src[:])
                return t

            ww_t = wtile(ww, [128, 128], "ww")
            et_t = wtile(et, [128, CHUNK], "et")
            redm_t = wtile(redm, [128, 2], "redm")
            redq_t = wtile(redq, [128, 2 * K], "redq")
            wa_t = [wtile(wa[k], [128, 128], f"wa{k}") for k in range(K)]
            wb_t = [wtile(wb[k], [128, 128], f"wb{k}") for k in range(K)]

            q = sp.tile([128, FREE], f32, tag="q")
            slw = sp.tile([2, FREE], f32, tag="slw")
            nc.vector.memset(slw[:], 0.0)

            def dma_in(dst, src):
                step = FREE // 4
                for j in range(4):
                    sl = slice(j * step, (j + 1) * step)
                    nc.sync.dma_start(dst[:, sl], src[:, sl])

            # q0 arrives via DRAM; its w-term is handled on host.
            dma_in(q, qn)

            for k in range(K):
                nz_t = npl.tile([128, FREE], f32, tag="nz")
                dma_in(nz_t, nz[k])
                sk = float(np.sqrt(db[k + 1]))
                for ci in range(NCH):
                    sl = slice(ci * CHUNK, (ci + 1) * CHUNK)
                    psq = pp.tile([128, CHUNK], f32, tag="psq")
                    nc.tensor.matmul(psq[:], wa_t[k][:], q[:, sl], start=True, stop=False)
                    nc.tensor.matmul(psq[:], wb_t[k][:], nz_t[:, sl], start=False, stop=True)
                    nc.vector.tensor_copy(q[:, sl], psq[:])

                    psr = pp.tile([128, CHUNK], f32, tag="psr")
                    nc.tensor.matmul(psr[:], ww_t[:], q[:, sl], start=True, stop=True)
                    rr = rp.tile([128, CHUNK], f32, tag="rr")
                    nc.scalar.activation(rr[:], psr[:], SQ, scale=sk)

                    qm = rp.tile([128, CHUNK], f32, tag="qm")
                    nc.gpsimd.tensor_mul(qm[:], q[:, sl], et_t[:])

                    # transposed reduce: red vectors stationary, rr/qm moving,
                    # both accumulate into one [2, CHUNK] psum; then one DVE
                    # add folds it into the SBUF slw accumulator.
                    psw = pw.tile([2, CHUNK], f32, tag="psw")
                    nc.tensor.matmul(psw[:], redm_t[:], rr[:], start=True, stop=False)
                    nc.tensor.matmul(
                        psw[:], redq_t[:, 2 * k : 2 * k + 2], qm[:],
                        start=False, stop=True,
                    )
                    nc.vector.tensor_add(slw[:, sl], slw[:, sl], psw[:])

            nc.sync.dma_start(out[:], slw[:])
    nc.compile()
    _cache["nc"] = nc
    return nc


def build_in_maps(x, W_enc, W_dec, q_noise):
    x = np.asarray(x, np.float32)
    hm = host_maps(x, W_enc, W_dec)
    beta, mu, W = hm["beta"], hm["mu"], hm["W"]
    db = beta[1:] - beta[:-1]
    nzl = _refresh_noise()
    q_noise = np.asarray(q_noise, np.float32)

    # q0 = mu + q_noise, in device layout
    q0l = _layout(q_noise) + _off_tile(mu)[None]
    offts = [_off_tile(o) for o in hm["offs"]]

    A = hm["A"]
    E = (A - mu)                                     # (B,D) f64
    eT = np.empty((2 * D, B), np.float32)
    eT[:D] = E.T.astype(np.float32)
    eT[D:] = E.T.astype(np.float32)
    et_np = np.tile(eT, (1, CHUNK // B))             # [128, 512]

    ww_np = _bd(W.astype(np.float32))

    half = np.zeros((128, 2), np.float32)
    half[:D, 0] = 1.0
    half[D:, 1] = 1.0
    redm_np = -0.5 * half
    redq_np = np.concatenate(
        [db[k + 1] * half for k in range(K)], axis=1
    ).astype(np.float32)                             # [128, 2K]

    wa_np = [_bd(a.astype(np.float32)) for a in hm["was"]]
    wb_np = [_bd(b.astype(np.float32)) for b in hm["wbs"]]

    in_maps = []
    for c in range(NCORES):
        m = {
            "qn": q0l[c],
            "ww": ww_np, "et": et_np,
            "redm": redm_np, "redq": redq_np,
        }
        for k in range(K):
            m[f"nz{k + 1}"] = nzl[k][c] + offts[k]
            m[f"wa{k + 1}"] = wa_np[k]
            m[f"wb{k + 1}"] = wb_np[k]
        in_maps.append(m)

    # host-side constants: k=0 incremental weight + per-step const terms
    q0 = mu[None].astype(np.float32) + q_noise       # (N,B,D)
    r0 = q0.reshape(-1, D).astype(np.float32) @ W.astype(np.float32)
    r0 = r0.reshape(N, B, DX) - x[None]
    w0 = (
        -0.5 * DX * LOG2PI
        - 0.5 * np.einsum("nbe,nbe->nb", r0, r0, dtype=np.float64)
        - np.einsum("nbd,bd->nb", q0, mu.astype(np.float32), dtype=np.float64)
        + 0.5 * (mu * mu).sum(-1)[None]
    )
    const_nb = db[0] * w0                            # (N,B) f64
    rest = beta[K + 1] - beta[1]                     # sum of db[1:]
    xd = x.astype(np.float64)
    const_b = rest * (
        -0.5 * DX * LOG2PI - 0.5 * (xd * xd).sum(-1) + 0.5 * (mu * mu).sum(-1)
    )  # (B,)
    const = (const_nb + const_b[None]).astype(np.float32)
    return in_maps, const


def decode_out(dev, c, slw_all):
    """dev [2=h, FREE=(s',b)] -> slw_all[c*NLOC:(c+1)*NLOC, :] (NLOC,B)."""
    slw_all[c * NLOC : (c + 1) * NLOC] = dev.reshape(NLOC, B)


def kernel(x, W_enc, W_dec, q_noise, p_noise):
    from concourse.bass_utils import run_bass_kernel_spmd

    nc = _program()
    in_maps, const = build_in_maps(x, W_enc, W_dec, q_noise)
    trace = bool(int(os.environ.get("KERNEL_TRACE", "0")))
    res = run_bass_kernel_spmd(nc, in_maps, list(range(NCORES)), trace=trace)
    _cache["last_result"] = res

    slw = np.empty((N, B), np.float32)
    for c in range(NCORES):
        decode_out(np.asarray(res.results[c]["out"]), c, slw)
    slw += const
    m = slw.max(0)
    out = m + np.log(np.exp(slw - m).sum(0, dtype=np.float32)) - math.log(float(N))
    return out.astype(np.float32)


# revision 12
# speedup vs baseline: 1.1807x; 1.0060x over previous
"""AIS estimator (nn_AIS_Estimator) on 8 Trainium2 NeuronCores.

Math: the reference's leapfrog gradient is affine, g(q) = C_k - q @ (b_k W W^T + I),
so each annealing step collapses (on host, f64) to an affine map
    q <- q @ A_k + n_k @ B_k + D_k        (A_k, B_k 64x64; D_k folded into the
                                           noise: n'_k = n_k + D_k @ inv(B_k))
The incremental weight is
    w(q) = -0.5*DX*LOG2PI - 0.5*||q W - x||^2 - q.mu + 0.5*||mu||^2
(prior - importance collapses to the bilinear -q.mu + const). The k=0 term
uses the raw inputs and is computed on host; steps 1..K run on device:
    psum_q = q @ A2_k + n' @ B2_k                      (PE)
    q      = copy(psum_q)                              (DVE)
    psum_r = q @ W2 + (-x) via identity-tile matmul    (PE)
    rr     = Square(sqrt(db_k) * psum_r)               (ACT)
    qm     = q * muT                                   (GPSIMD)
    slw   += reduce_partitions(rr, -0.5) + reduce(qm, -db_k)   (PE, rr/qm as
             stationary operand, [128,2] out per 128-col sub-tile into a
             single [128,128] PSUM accumulator)
Momentum refresh noise is input-independent (fold_in(key42, k)) and
precomputed on host CPU via JAX threefry.

Device layout per core (128 of 1024 samples, pure data parallel):
  partitions = (half, d) [2*64], free = (s', b) [64*128 = 8192]; 64-dim
  matmuls use all 128 partitions via block-diagonal weights. Host gathers
  per-core [128,128] slw partials, adds constants, does logsumexp over n.
"""

import math
import os

import numpy as np

N, B, D, DX = 1024, 128, 64, 64
K, NL, H = 16, 3, 0.05
NCORES = 8
NLOC = N // NCORES          # samples per core
SP = NLOC // 2              # s' per half
FREE = SP * B               # 8192
CHUNK = 512
NCH = FREE // CHUNK         # 16
LOG2PI = math.log(2.0 * math.pi)

_cache = {}


def _betas():
    z = 4.0 * (2.0 * np.linspace(0.0, 1.0, K + 2) - 1.0)
    b = 1.0 / (1.0 + np.exp(-z))
    return (b - b[0]) / (b[-1] - b[0])  # f64, [K+2]


def _layout(arr):
    """(N,B,D) f32 -> [NCORES, 128, FREE] device layout [p=(h,d), f=(s',b)]."""
    a = arr.reshape(NCORES, 2, SP, B, D).transpose(0, 1, 4, 2, 3)
    return np.ascontiguousarray(a.reshape(NCORES, 2 * D, FREE))


def _off_tile(off):
    """(B,D) f64 offset -> [128, FREE] f32 tile broadcast over s'."""
    t = np.empty((2 * D, B), np.float32)
    t[:D] = off.T.astype(np.float32)
    t[D:] = off.T.astype(np.float32)
    return np.tile(t, (1, SP))


def _bd(m):
    """(D,D) -> [128,128] f32 block-diag (two sample-halves)."""
    r, c = m.shape
    out = np.zeros((2 * r, 2 * c), np.float32)
    out[:r, :c] = m
    out[r:, c:] = m
    return out


def _dup_cols(m):
    """(B, D) -> [128, 128] f32: columns duplicated for both halves."""
    out = np.empty((B, 2 * m.shape[1]), np.float32)
    out[:, : m.shape[1]] = m
    out[:, m.shape[1] :] = m
    return out


def _refresh_noise():
    # Momentum refresh noise: jax.random.normal(fold_in(key(42), k), ...).
    # Generated on the DEFAULT jax backend: with jax_default_prng_impl=rbg the
    # bits are backend-dependent, and the graded reference runs on the default
    # backend, so we must sample from the same generator.
    if "nz" not in _cache:
        import jax
        import jax.numpy as jnp

        gen = jax.jit(
            lambda k: jax.random.normal(
                jax.random.fold_in(jax.random.key(42), k), (N, B, D), jnp.float32
            )
        )
        out = [_layout(np.asarray(gen(jnp.int32(k)))) for k in range(1, K + 1)]
        _cache["nz"] = out
    return _cache["nz"]


def host_maps(x, W_enc, W_dec):
    """Input-dependent f64 precomputation of the per-step affine maps."""
    xd = np.asarray(x, np.float64)
    Wed = np.asarray(W_enc, np.float64)
    Wdd = np.asarray(W_dec, np.float64)
    beta = _betas()
    mu = xd @ Wed                      # (B,D)
    A = xd @ Wdd.T                     # (B,D)
    M = Wdd @ Wdd.T                    # A is reused for the q.E bilinear
    I = np.eye(D)

    was, wbs, offs = [], [], []
    for k in range(1, K + 1):
        bk = beta[k]
        Mk = bk * M + I
        Ck = bk * A + (1.0 - bk) * mu
        al, be = I.copy(), np.zeros((D, D))
        ga, ep = np.zeros((D, D)), H * I
        de, ze = np.zeros((B, D)), np.zeros((B, D))
        for j in range(NL):
            s = H * (H / 2 if j == 0 else H)
            ga = ga - s * (al @ Mk)
            ep = ep - s * (be @ Mk)
            ze = ze + s * Ck - s * (de @ Mk)
            al = al + ga
            be = be + ep
            de = de + ze
        was.append(al)
        wbs.append(be)
        offs.append(de @ np.linalg.inv(be))

    return dict(was=was, wbs=wbs, offs=offs, mu=mu, W=Wdd, beta=beta, A=A)


def _program():
    if "nc" in _cache:
        return _cache["nc"]
    import concourse.bacc as bacc
    import concourse.mybir as mybir
    from concourse import tile

    f32 = mybir.dt.float32
    SQ = mybir.ActivationFunctionType.Square
    beta = _betas()
    db = beta[1:] - beta[:-1]

    nc = bacc.Bacc()
    qn = nc.declare_dram_parameter("qn", [128, FREE], f32, isOutput=False)
    nz = [
        nc.declare_dram_parameter(f"nz{k}", [128, FREE], f32, isOutput=False)
        for k in range(1, K + 1)
    ]
    wa = [
        nc.declare_dram_parameter(f"wa{k}", [128, 128], f32, isOutput=False)
        for k in range(1, K + 1)
    ]
    wb = [
        nc.declare_dram_parameter(f"wb{k}", [128, 128], f32, isOutput=False)
        for k in range(1, K + 1)
    ]
    ww = nc.declare_dram_parameter("ww", [128, 128], f32, isOutput=False)
    et = nc.declare_dram_parameter("et", [128, CHUNK], f32, isOutput=False)
    redm = nc.declare_dram_parameter("redm", [128, 2], f32, isOutput=False)
    redq = nc.declare_dram_parameter("redq", [128, 2 * K], f32, isOutput=False)
    out = nc.declare_dram_parameter("out", [2, FREE], f32, isOutput=True)

    with tile.TileContext(nc) as tc:
        with (
            tc.tile_pool(name="wts", bufs=1) as wp,
            tc.tile_pool(name="state", bufs=1) as sp,
            tc.tile_pool(name="noise", bufs=2) as npl,
            tc.tile_pool(name="rr", bufs=3) as rp,
            tc.tile_pool(name="ps", bufs=4, space="PSUM") as pp,
            tc.tile_pool(name="psr", bufs=2, space="PSUM") as pr,
            tc.tile_pool(name="psw", bufs=2, space="PSUM") as pw,
        ):
            def wtile(src, shape, tag):
                t = wp.tile(shape, f32, tag=tag)
                nc.sync.dma_start(t[:], src[:])
                return t

            ww_t = wtile(ww, [128, 128], "ww")
            et_t = wtile(et, [128, CHUNK], "et")
            redm_t = wtile(redm, [128, 2], "redm")
            redq_t = wtile(redq, [128, 2 * K], "redq")
            wa_t = [wtile(wa[k], [128, 128], f"wa{k}") for k in range(K)]
            wb_t = [wtile(wb[k], [128, 128], f"wb{k}") for k in range(K)]

            q = sp.tile([128, FREE], f32, tag="q")
            slw = sp.tile([2, FREE], f32, tag="slw")
            nc.vector.memset(slw[:], 0.0)

            def dma_in(dst, src):
                step = FREE // 4
                for j in range(4):
                    sl = slice(j * step, (j + 1) * step)
                    nc.sync.dma_start(dst[:, sl], src[:, sl])

            # q0 arrives via DRAM; its w-term is handled on host.
            dma_in(q, qn)

            # Global software pipeline over the 256 (step, chunk) iterations:
            # the recurrence matmuls for chunk t+LOOK are emitted between the
            # w-matmul and the reduce matmuls of chunk t, so PE never stalls
            # on the DVE copy -> ACT square chain.
            LOOK = 3
            TOT = K * NCH
            psq_t = [None] * TOT
            nz_tiles = {}

            def emit_front(t):
                k, ci = divmod(t, NCH)
                if ci == 0:
                    nz_t = npl.tile([128, FREE], f32, tag="nz")
                    dma_in(nz_t, nz[k])
                    nz_tiles[k] = nz_t
                    if k - 2 in nz_tiles:
                        del nz_tiles[k - 2]
                sl = slice(ci * CHUNK, (ci + 1) * CHUNK)
                psq = pp.tile([128, CHUNK], f32, tag="psq")
                nc.tensor.matmul(psq[:], wa_t[k][:], q[:, sl], start=True, stop=False)
                nc.tensor.matmul(psq[:], wb_t[k][:], nz_tiles[k][:, sl], start=False, stop=True)
                nc.vector.tensor_copy(q[:, sl], psq[:])
                psq_t[t] = psq

            def emit_mid(t):
                k, ci = divmod(t, NCH)
                sl = slice(ci * CHUNK, (ci + 1) * CHUNK)
                qm = rp.tile([128, CHUNK], f32, tag="qm")
                nc.gpsimd.tensor_mul(qm[:], q[:, sl], et_t[:])
                psr = pr.tile([128, CHUNK], f32, tag="psr")
                nc.tensor.matmul(psr[:], ww_t[:], q[:, sl], start=True, stop=True)
                rr = rp.tile([128, CHUNK], f32, tag="rr")
                sk = float(np.sqrt(db[k + 1]))
                nc.scalar.activation(rr[:], psr[:], SQ, scale=sk)
                return k, sl, rr, qm

            def emit_back(state):
                k, sl, rr, qm = state
                psw = pw.tile([2, CHUNK], f32, tag="psw")
                nc.tensor.matmul(psw[:], redm_t[:], rr[:], start=True, stop=False)
                nc.tensor.matmul(
                    psw[:], redq_t[:, 2 * k : 2 * k + 2], qm[:],
                    start=False, stop=True,
                )
                nc.vector.tensor_add(slw[:, sl], slw[:, sl], psw[:])

            pending = None
            for t in range(TOT + LOOK):
                if pending is not None:
                    mid_state = emit_mid(pending)
                else:
                    mid_state = None
                if t < TOT:
                    emit_front(t)
                if mid_state is not None:
                    emit_back(mid_state)
                pending = t - LOOK + 1 if t - LOOK + 1 >= 0 and t - LOOK + 1 < TOT else None

            nc.sync.dma_start(out[:], slw[:])
    nc.compile()
    _cache["nc"] = nc
    return nc


def build_in_maps(x, W_enc, W_dec, q_noise):
    x = np.asarray(x, np.float32)
    hm = host_maps(x, W_enc, W_dec)
    beta, mu, W = hm["beta"], hm["mu"], hm["W"]
    db = beta[1:] - beta[:-1]
    nzl = _refresh_noise()
    q_noise = np.asarray(q_noise, np.float32)

    # q0 = mu + q_noise, in device layout
    q0l = _layout(q_noise) + _off_tile(mu)[None]
    offts = [_off_tile(o) for o in hm["offs"]]

    A = hm["A"]
    E = (A - mu)                                     # (B,D) f64
    eT = np.empty((2 * D, B), np.float32)
    eT[:D] = E.T.astype(np.float32)
    eT[D:] = E.T.astype(np.float32)
    et_np = np.tile(eT, (1, CHUNK // B))             # [128, 512]

    ww_np = _bd(W.astype(np.float32))

    half = np.zeros((128, 2), np.float32)
    half[:D, 0] = 1.0
    half[D:, 1] = 1.0
    redm_np = -0.5 * half
    redq_np = np.concatenate(
        [db[k + 1] * half for k in range(K)], axis=1
    ).astype(np.float32)                             # [128, 2K]

    wa_np = [_bd(a.astype(np.float32)) for a in hm["was"]]
    wb_np = [_bd(b.astype(np.float32)) for b in hm["wbs"]]

    in_maps = []
    for c in range(NCORES):
        m = {
            "qn": q0l[c],
            "ww": ww_np, "et": et_np,
            "redm": redm_np, "redq": redq_np,
        }
        for k in range(K):
            m[f"nz{k + 1}"] = nzl[k][c] + offts[k]
            m[f"wa{k + 1}"] = wa_np[k]
            m[f"wb{k + 1}"] = wb_np[k]
        in_maps.append(m)

    # host-side constants: k=0 incremental weight + per-step const terms
    q0 = mu[None].astype(np.float32) + q_noise       # (N,B,D)
    r0 = q0.reshape(-1, D).astype(np.float32) @ W.astype(np.float32)
    r0 = r0.reshape(N, B, DX) - x[None]
    w0 = (
        -0.5 * DX * LOG2PI
        - 0.5 * np.einsum("nbe,nbe->nb", r0, r0, dtype=np.float64)
        - np.einsum("nbd,bd->nb", q0, mu.astype(np.float32), dtype=np.float64)
        + 0.5 * (mu * mu).sum(-1)[None]
    )
    const_nb = db[0] * w0                            # (N,B) f64
    rest = beta[K + 1] - beta[1]                     # sum of db[1:]
    xd = x.astype(np.float64)
    const_b = rest * (
        -0.5 * DX * LOG2PI - 0.5 * (xd * xd).sum(-1) + 0.5 * (mu * mu).sum(-1)
    )  # (B,)
    const = (const_nb + const_b[None]).astype(np.float32)
    return in_maps, const


def decode_out(dev, c, slw_all):
    """dev [2=h, FREE=(s',b)] -> slw_all[c*NLOC:(c+1)*NLOC, :] (NLOC,B)."""
    slw_all[c * NLOC : (c + 1) * NLOC] = dev.reshape(NLOC, B)


def kernel(x, W_enc, W_dec, q_noise, p_noise):
    from concourse.bass_utils import run_bass_kernel_spmd

    nc = _program()
    in_maps, const = build_in_maps(x, W_enc, W_dec, q_noise)
    trace = bool(int(os.environ.get("KERNEL_TRACE", "0")))
    res = run_bass_kernel_spmd(nc, in_maps, list(range(NCORES)), trace=trace)
    _cache["last_result"] = res

    slw = np.empty((N, B), np.float32)
    for c in range(NCORES):
        decode_out(np.asarray(res.results[c]["out"]), c, slw)
    slw += const
    m = slw.max(0)
    out = m + np.log(np.exp(slw - m).sum(0, dtype=np.float32)) - math.log(float(N))
    return out.astype(np.float32)
